# revision 37
# baseline (speedup 1.0000x reference)
"""Trainium2 Bass kernel for nn_Attention_74088185856351.

Strategy: data-parallel over batch (8 batches -> 8 NeuronCores), full
attention per core. Host pre-transposes x / weights so the device does
only matmuls, norm stats, exp.

Per-core pipeline (N=1024 tokens, C=768, H=12 heads, D=64):
  M1   : qkv = xT.T @ qkv_w.T     (bf16 matmuls, fp32 PSUM accum)
  norm : per-head standardization of q,k (ddof=1), fused with PSUM
         evacuation; outputs bf16
  qT/kT: DMA-transpose (bf16) to [d, n] layout
  M2   : logitsT[nk,nq] per head  (bf16, K=64)
  exp  : ACT Exp with 1/8 scale folded, PSUM->SBUF bf16
  M3   : outT'[65,nq] = [v|1].T @ expT  -> row 64 = softmax denom S
  evac : out = outT/S  (fast-approx reciprocal + gpsimd
         partition_broadcast + mult)
  proj : y = attnoutT.T @ proj_w.T + bias (bf16 matmul, fp32 out)
"""
import sys
sys.path.insert(0, '/opt/trn_rl_repo')
import numpy as np

B, N, C, H = 8, 1024, 768, 12
D = C // H          # 64
NP = N // 128       # 8 n-chunks
KC = C // 128       # 6 contraction chunks
G = 6               # heads per 384-wide M1 chunk

_CACHE = {}


def _build_nc():
    import concourse.bacc as bacc
    import concourse.tile as tile
    from concourse import mybir

    F32, BF16 = mybir.dt.float32, mybir.dt.bfloat16
    AX, ALU, AF = mybir.AxisListType, mybir.AluOpType, mybir.ActivationFunctionType

    nc = bacc.Bacc("TRN2", target_bir_lowering=False, debug=False, num_devices=8)
    xT_d = nc.declare_dram_parameter("xT", [C, N], BF16, isOutput=False)
    wq_d = nc.declare_dram_parameter("wqkvT", [C, 3 * C], BF16, isOutput=False)
    wp_d = nc.declare_dram_parameter("wpT", [C, C], BF16, isOutput=False)
    bias_d = nc.declare_dram_parameter("bias", [1, C], F32, isOutput=False)
    ones_d = nc.declare_dram_parameter("ones16", [128, 16], BF16, isOutput=False)
    ident_d = nc.declare_dram_parameter("ident", [128, 128], BF16, isOutput=False)
    # per-head column sums of the q/k weights: head-sums of qkv come from
    # a tiny F=24 matmul instead of two DVE reductions per norm chunk
    wsum_d = nc.declare_dram_parameter("wsumT", [C, 24], BF16, isOutput=False)
    y_d = nc.declare_dram_parameter("y", [N, C], F32, isOutput=True)

    # M1 column-chunk consumption order: v chunks first, then q/k pairs
    FC_ORDER = (4, 5, 0, 2, 1, 3)

    with tile.TileContext(nc) as tc:
        with (
            tc.tile_pool(name="sba", bufs=1) as sba,
            tc.tile_pool(name="sbt", bufs=5) as sbt,
        ):
            sbw_cm = tc.tile_pool(name="sbw", bufs=1)
            sbw = sbw_cm.__enter__()
            # ---- loads (in consumption order; v-phase weights on the
            # scalar queue so they land while xT streams on sync) ----
            # critical first loads (xT + v-phase weights) round-robin all
            # 3 DMA-capable queues; later loads stay off the scalar queue
            # so ACT compute is never stuck behind descriptor generation.
            rr = [0]
            ENGS = (nc.sync, nc.scalar, nc.gpsimd)

            def load(eng_set, out, in_):
                eng_set[rr[0] % len(eng_set)].dma_start(out=out, in_=in_)
                rr[0] += 1

            # xT as separate half tiles: n-chunks 0-3 in xTh[0][k], 4-7 in
            # xTh[1][k] — lets the v-phase start on ~1.9MB instead of 8.5MB
            xTh = [[sbw.tile([128, 512], BF16, tag=f"xT{h}_{k}", name=f"xT{h}_{k}")
                    for k in range(KC)] for h in range(2)]
            for k in range(KC):
                load(ENGS, xTh[0][k], xT_d[k * 128:(k + 1) * 128, 0:512])
            wq = {}
            for fc in (4, 5):
                for k in range(KC):
                    t = sbw.tile([128, 384], BF16, tag=f"wq{fc}_{k}", name=f"wq{fc}_{k}")
                    load(ENGS, t, wq_d[k * 128:(k + 1) * 128, fc * 384:(fc + 1) * 384])
                    wq[(fc, k)] = t
            for k in range(KC):
                load(ENGS, xTh[1][k], xT_d[k * 128:(k + 1) * 128, 512:1024])

            def xt_ap(k, n):
                return xTh[n // 4][k][:, (n % 4) * 128:(n % 4 + 1) * 128]
            wsum = [sbw.tile([128, 24], BF16, tag=f"ws{k}", name=f"ws{k}") for k in range(KC)]
            for k in range(KC):
                load((nc.sync, nc.gpsimd), wsum[k], wsum_d[k * 128:(k + 1) * 128, :])
            for fc in (0, 2, 1, 3):
                for k in range(KC):
                    t = sbw.tile([128, 384], BF16, tag=f"wq{fc}_{k}", name=f"wq{fc}_{k}")
                    load((nc.sync, nc.gpsimd), t, wq_d[k * 128:(k + 1) * 128, fc * 384:(fc + 1) * 384])
                    wq[(fc, k)] = t
            wp = [sba.tile([128, C], BF16, tag=f"wp{k}", name=f"wp{k}") for k in range(KC)]
            for k in range(KC):
                load((nc.sync, nc.gpsimd), wp[k], wp_d[k * 128:(k + 1) * 128, :])
            import concourse.bass as bass
            bias_sb = sba.tile([128, C], F32, tag="bias")
            bias_bcast = bass.AP(tensor=bias_d.tensor if hasattr(bias_d, 'tensor') else bias_d,
                                 offset=0, ap=[[0, 128], [1, C]])
            nc.gpsimd.dma_start(out=bias_sb, in_=bias_bcast)
            ones_sb = sba.tile([128, 16], BF16, tag="ones")
            nc.sync.dma_start(out=ones_sb, in_=ones_d[:, :])
            ident_sb = sba.tile([128, 128], BF16, tag="ident")
            nc.sync.dma_start(out=ident_sb, in_=ident_d[:, :])

            # persistent attention-side tensors.  qkT interleaves q/k head
            # pairs as 1024-col blocks [q0 k0 q1 k1 ...] so one strided copy
            # evacuates a whole norm-chunk's 6 transposes.
            qkT = sba.tile([128, 12 * N], BF16, tag="qkT")
            v1 = [sba.tile([128, H, D + 1], BF16, tag=f"v1{n}", name=f"v1{n}") for n in range(NP)]
            aoT = [sba.tile([128, N], BF16, tag=f"aoT{p}", name=f"aoT{p}") for p in range(6)]

            def m1_chunk(pool, fc, n):
                """One M1 accumulation: psum[128,384] = x-chunk @ w-cols."""
                pt = pool.tile([128, 384], F32, tag="m1", name="m1ps")
                for k in range(KC):
                    nc.tensor.matmul(
                        pt[:, :],
                        xt_ap(k, n),
                        wq[(fc, k)][:, :],
                        start=(k == 0), stop=(k == KC - 1))
                return pt

            def evac_v(fc, n, pt):
                g0 = (fc - 4) * G
                nc.scalar.copy(
                    v1[n][:, g0:g0 + G, 0:D],
                    pt[:, :].rearrange("p (g d) -> p g d", g=G))
                if fc == 5:
                    nc.vector.tensor_copy(
                        v1[n][:, :, D:D + 1].rearrange("p h one -> p (h one)"),
                        ones_sb[:, 0:H])

            tx_count = [0]

            def evac_qk(pool, psT, fcq, fck, n, sums):
                """Normalize 6 q-heads + 6 k-heads (same n-slice) jointly.
                `sums` is a [128,12] AP of per-head sums (q6|k6), computed
                on the PE from host-pre-summed weight columns.

                Returns a closure emitting the PE transposes + evac copies,
                so the caller can software-pipeline them one iteration
                behind (keeps the in-order PE from stalling on norm)."""
                ptq = m1_chunk(pool, fcq, n)
                ptk = m1_chunk(pool, fck, n)
                G2 = 2 * G
                sq = sbt.tile([128, 768], F32, tag="sq")
                sumsq = sbt.tile([128, G2], F32, tag="sumsq")
                for i, pt in enumerate((ptq, ptk)):
                    nc.scalar.activation(out=sq[:, i * 384:(i + 1) * 384],
                                         in_=pt[:, :], func=AF.Square)
                nc.vector.tensor_reduce(
                    out=sumsq, in_=sq[:, :].rearrange("p (g d) -> p g d", g=G2),
                    axis=AX.X, op=ALU.add)
                mean = sbt.tile([128, G2], F32, tag="mean")
                nc.gpsimd.tensor_scalar(out=mean, in0=sums, scalar1=1.0 / D,
                                        scalar2=None, op0=ALU.mult)
                var63 = sbt.tile([128, G2], F32, tag="var63")
                nc.gpsimd.tensor_scalar(out=var63, in0=sums, scalar1=-1.0 / D,
                                        scalar2=None, op0=ALU.mult)
                nc.gpsimd.tensor_tensor(out=var63, in0=var63, in1=sums, op=ALU.mult)
                nc.gpsimd.tensor_tensor(out=var63, in0=var63, in1=sumsq, op=ALU.add)
                rstd = sbt.tile([128, G2], F32, tag="rstd")
                nc.scalar.activation(out=rstd, in_=var63, func=AF.Sqrt,
                                     scale=1.0 / (D - 1))
                nc.vector.reciprocal(out=rstd, in_=rstd)
                nmr = sbt.tile([128, G2], F32, tag="nmr")
                nc.gpsimd.tensor_tensor(out=nmr, in0=mean, in1=rstd, op=ALU.mult)
                nc.gpsimd.tensor_scalar(out=nmr, in0=nmr, scalar1=-1.0,
                                        scalar2=None, op0=ALU.mult)
                qkns = []
                for i, pt in enumerate((ptq, ptk)):
                    grp = pt[:, :].rearrange("p (g d) -> p g d", g=G)
                    qkn = sbt.tile([128, 384], BF16, tag=f"qkn{i}", name=f"qkn{i}")
                    qkng = qkn[:, :].rearrange("p (g d) -> p g d", g=G)
                    for g in range(G):
                        gg = i * G + g
                        if g % 2 == 0:
                            nc.vector.tensor_scalar(
                                out=qkng[:, g, :], in0=grp[:, g, :],
                                scalar1=mean[:, gg:gg + 1], scalar2=rstd[:, gg:gg + 1],
                                op0=ALU.subtract, op1=ALU.mult)
                        else:
                            nc.scalar.activation(out=qkng[:, g, :], in_=grp[:, g, :],
                                                 func=AF.Identity, bias=nmr[:, gg:gg + 1],
                                                 scale=rstd[:, gg:gg + 1])
                    qkns.append(qkn)

                def emit_transposes():
                    bp2 = 0 if fcq in (0, 2) else 6
                    ptt = psT.tile([128, 768], BF16, tag="tp", name="tpps")
                    for j in range(3):
                        nc.tensor.transpose(
                            ptt[:, (2 * j) * 128:(2 * j + 1) * 128],
                            qkns[0][:, j * 128:(j + 1) * 128], ident_sb[:, :])
                        nc.tensor.transpose(
                            ptt[:, (2 * j + 1) * 128:(2 * j + 2) * 128],
                            qkns[1][:, j * 128:(j + 1) * 128], ident_sb[:, :])
                    dst = qkT[:, :].rearrange(
                        "p (b n c) -> p b n c", b=12, n=NP)[:, bp2:bp2 + 6, n, :]
                    nc.vector.tensor_copy(dst, ptt[:, :].rearrange("p (j c) -> p j c", j=6))
                return emit_transposes

            def attention_pair(p, sbe, ps2, ps3):
                for par in range(2):
                    h = 2 * p + par
                    et = [sbe.tile([128, N], BF16, tag=f"e{par}{nk}", name=f"e{par}{nk}") for nk in range(NP)]
                    qb, kb = (2 * p) * N, (2 * p + 1) * N
                    for nk in range(NP):
                        p2 = ps2.tile([128, 1024], F32, tag="m2", name="m2ps")
                        for nqh in range(2):
                            nc.tensor.matmul(
                                p2[:, nqh * 512:(nqh + 1) * 512],
                                qkT[par * D:(par + 1) * D, kb + nk * 128:kb + (nk + 1) * 128],
                                qkT[par * D:(par + 1) * D, qb + nqh * 512:qb + (nqh + 1) * 512],
                                start=True, stop=True)
                        nc.scalar.activation(
                            out=et[nk][:, :],
                            in_=p2[:, :], func=AF.Exp, scale=float(D) ** -0.5)
                    for nqh in range(2):
                        p3 = ps3.tile([D + 1, 512], F32, tag="m3", name="m3ps")
                        for nk in range(NP):
                            nc.tensor.matmul(
                                p3[:, :],
                                v1[nk][:, h, :],
                                et[nk][:, nqh * 512:(nqh + 1) * 512],
                                start=(nk == 0), stop=(nk == NP - 1))
                        sS = sbt.tile([1, 512], F32, tag="sS")
                        nc.vector.tensor_copy(sS[:, :], p3[D:D + 1, :])
                        rS = sbt.tile([1, 512], F32, tag="rS")
                        nc.vector.reciprocal_approx_fast(out=rS, in_=sS[:, :])
                        rSb = sbt.tile([D, 512], F32, tag="rSb")
                        nc.gpsimd.partition_broadcast(rSb[:, :], rS[:, :])
                        nc.vector.tensor_tensor(
                            out=aoT[p][par * D:(par + 1) * D,
                                       nqh * 512:(nqh + 1) * 512],
                            in0=p3[0:D, :], in1=rSb[:, :], op=ALU.mult)

            # ---------- phase 1: M1 + norm ----------
            ps1_cm = tc.tile_pool(name="ps1", bufs=5, space="PSUM")
            ps1 = ps1_cm.__enter__()
            psT_cm = tc.tile_pool(name="psT", bufs=2, space="PSUM")
            psT = psT_cm.__enter__()
            psS_cm = tc.tile_pool(name="psS", bufs=1, space="PSUM")
            psS = psS_cm.__enter__()
            for fc in (4, 5):                      # v first
                for n in range(NP):
                    evac_v(fc, n, m1_chunk(ps1, fc, n))
            sums_n = [sba.tile([128, 24], F32, tag=f"sums{n}", name=f"sums{n}")
                      for n in range(NP)]
            pending = None
            for fcq, fck in ((0, 2), (1, 3)):      # q,k (sqrt-table phase)
                for n in range(NP):
                    if fcq == 0:                   # head-sums, reused by pass 2
                        pss = psS.tile([128, 24], F32, tag="ws", name="wsps")
                        for k in range(KC):
                            nc.tensor.matmul(
                                pss[:, :], xt_ap(k, n),
                                wsum[k][:, :], start=(k == 0), stop=(k == KC - 1))
                        nc.vector.tensor_copy(sums_n[n][:, :], pss[:, :])
                    off = 0 if fcq == 0 else 12
                    w = evac_qk(ps1, psT, fcq, fck, n, sums_n[n][:, off:off + 12])
                    if pending is not None:
                        pending()
                    pending = w
            pending()
            psS_cm.__exit__(None, None, None)
            psT_cm.__exit__(None, None, None)
            ps1_cm.__exit__(None, None, None)
            sbw_cm.__exit__(None, None, None)      # free x/w region for expT

            # ---------- phase 2: attention (exp-table phase) ----------
            sbe_cm = tc.tile_pool(name="sbe", bufs=2)
            sbe = sbe_cm.__enter__()
            ps2_cm = tc.tile_pool(name="ps2", bufs=2, space="PSUM")
            ps2 = ps2_cm.__enter__()
            ps3_cm = tc.tile_pool(name="ps3", bufs=4, space="PSUM")
            ps3 = ps3_cm.__enter__()
            for p in range(6):
                attention_pair(p, sbe, ps2, ps3)
            ps3_cm.__exit__(None, None, None)
            ps2_cm.__exit__(None, None, None)
            sbe_cm.__exit__(None, None, None)

            # ---------- phase 3: proj ----------
            psP_cm = tc.tile_pool(name="psP", bufs=4, space="PSUM")
            psP = psP_cm.__enter__()
            for n in range(NP):
                ysb = sbt.tile([128, C], F32, tag="y")
                for half in range(2):
                    pp = psP.tile([128, 384], F32, tag="mp", name="mpps")
                    for k in range(KC):
                        nc.tensor.matmul(
                            pp[:, :],
                            aoT[k][:, n * 128:(n + 1) * 128],
                            wp[k][:, half * 384:(half + 1) * 384],
                            start=(k == 0), stop=(k == KC - 1))
                    nc.vector.tensor_tensor(
                        out=ysb[:, half * 384:(half + 1) * 384], in0=pp[:, :],
                        in1=bias_sb[:, half * 384:(half + 1) * 384], op=ALU.add)
                    nc.sync.dma_start(
                        out=y_d[n * 128:(n + 1) * 128, half * 384:(half + 1) * 384],
                        in_=ysb[:, half * 384:(half + 1) * 384])
            psP_cm.__exit__(None, None, None)

    nc.compile()
    return nc


def _prep_inputs(x, qkv_w, proj_w, proj_b):
    import ml_dtypes
    wqkvT = np.ascontiguousarray(qkv_w.T).astype(ml_dtypes.bfloat16)    # [768, 2304]
    wpT = np.ascontiguousarray(proj_w.T).astype(ml_dtypes.bfloat16)     # [768, 768]
    bias = proj_b.reshape(1, C).astype(np.float32)
    ones16 = np.ones((128, 16), dtype=ml_dtypes.bfloat16)
    ident = np.eye(128, dtype=ml_dtypes.bfloat16)
    # per-head column sums of q/k weights, ordered [q0-5, k0-5, q6-11, k6-11]
    # to match the per-pair (fcq, fck) consumption layout
    ws = wqkvT[:, :2 * C].astype(np.float64).reshape(C, 24, D).sum(-1)  # [768, 24]
    order = list(range(0, 6)) + list(range(12, 18)) + list(range(6, 12)) + list(range(18, 24))
    wsumT = np.ascontiguousarray(ws[:, order]).astype(ml_dtypes.bfloat16)
    maps = []
    for b in range(B):
        maps.append({
            "xT": np.ascontiguousarray(x[b].T).astype(ml_dtypes.bfloat16),
            "wqkvT": wqkvT, "wpT": wpT, "bias": bias, "ones16": ones16,
            "ident": ident, "wsumT": wsumT,
        })
    return maps


def kernel(x, qkv_w, proj_w, proj_b):
    from concourse.bass_utils import run_bass_kernel_spmd
    if "nc" not in _CACHE:
        _CACHE["nc"] = _build_nc()
    nc = _CACHE["nc"]
    maps = _prep_inputs(np.asarray(x), np.asarray(qkv_w),
                        np.asarray(proj_w), np.asarray(proj_b))
    import os
    trace = bool(os.environ.get("KERNEL_TRACE"))
    res = run_bass_kernel_spmd(nc, maps, list(range(B)), trace=trace)
    _CACHE["last_result"] = res
    out = np.stack([res.results[b]["y"] for b in range(B)], axis=0)
    return out.astype(np.float32)


# revision 46
# speedup vs baseline: 1.3226x; 1.3226x over previous
"""Trainium2 Bass kernel for nn_Attention_74088185856351.

Strategy: data-parallel over batch (8 batches -> 8 NeuronCores), full
attention per core. Host pre-transposes x / weights so the device does
only matmuls, norm stats, exp.

Per-core pipeline (N=1024 tokens, C=768, H=12 heads, D=64):
  M1   : qkv = xT.T @ qkv_w.T     (bf16 matmuls, fp32 PSUM accum)
  norm : per-head standardization of q,k (ddof=1), fused with PSUM
         evacuation; outputs bf16
  qT/kT: DMA-transpose (bf16) to [d, n] layout
  M2   : logitsT[nk,nq] per head  (bf16, K=64)
  exp  : ACT Exp with 1/8 scale folded, PSUM->SBUF bf16
  M3   : outT'[65,nq] = [v|1].T @ expT  -> row 64 = softmax denom S
  evac : out = outT/S  (fast-approx reciprocal + gpsimd
         partition_broadcast + mult)
  proj : y = attnoutT.T @ proj_w.T + bias (bf16 matmul, fp32 out)
"""
import sys
sys.path.insert(0, '/opt/trn_rl_repo')
import numpy as np

B, N, C, H = 8, 1024, 768, 12
D = C // H          # 64
NP = N // 128       # 8 n-chunks
KC = C // 128       # 6 contraction chunks
G = 6               # heads per 384-wide M1 chunk

_CACHE = {}


def _build_nc():
    import concourse.bacc as bacc
    import concourse.tile as tile
    from concourse import mybir

    F32, BF16 = mybir.dt.float32, mybir.dt.bfloat16
    AX, ALU, AF = mybir.AxisListType, mybir.AluOpType, mybir.ActivationFunctionType

    nc = bacc.Bacc("TRN2", target_bir_lowering=False, debug=False, num_devices=8)
    xT_d = nc.declare_dram_parameter("xT", [C, N], BF16, isOutput=False)
    wq_d = nc.declare_dram_parameter("wqkvT", [C, 3 * C], BF16, isOutput=False)
    wp_d = nc.declare_dram_parameter("wpT", [C, C], BF16, isOutput=False)
    bias_d = nc.declare_dram_parameter("bias", [1, C], F32, isOutput=False)
    ones_d = nc.declare_dram_parameter("ones16", [128, 16], BF16, isOutput=False)
    ident_d = nc.declare_dram_parameter("ident", [128, 128], BF16, isOutput=False)
    # q/k weight chunks augmented with 6 per-head column-sum columns
    # ([C, 4, 390], fc order 0,2,1,3): head-sums fall out of the same M1
    # matmul at F-cost +6 instead of a DVE reduction per chunk
    wqx_d = nc.declare_dram_parameter("wqxT", [C, 4 * 390], BF16, isOutput=False)
    y_d = nc.declare_dram_parameter("y", [N, C], F32, isOutput=True)

    # M1 column-chunk consumption order: v chunks first, then q/k pairs
    FC_ORDER = (4, 5, 0, 2, 1, 3)

    with tile.TileContext(nc) as tc:
        with (
            tc.tile_pool(name="sba", bufs=1) as sba,
            tc.tile_pool(name="sbt", bufs=5) as sbt,
        ):
            sbw_cm = tc.tile_pool(name="sbw", bufs=1)
            sbw = sbw_cm.__enter__()
            # ---- loads (in consumption order; v-phase weights on the
            # scalar queue so they land while xT streams on sync) ----
            # critical first loads (xT + v-phase weights) round-robin all
            # 3 DMA-capable queues; later loads stay off the scalar queue
            # so ACT compute is never stuck behind descriptor generation.
            rr = [0]
            ENGS = (nc.sync, nc.scalar, nc.gpsimd)

            def load(eng_set, out, in_):
                eng_set[rr[0] % len(eng_set)].dma_start(out=out, in_=in_)
                rr[0] += 1

            # xT as separate half tiles: n-chunks 0-3 in xTh[0][k], 4-7 in
            # xTh[1][k] — lets the v-phase start on ~1.9MB instead of 8.5MB
            xTh = [[sbw.tile([128, 512], BF16, tag=f"xT{h}_{k}", name=f"xT{h}_{k}")
                    for k in range(KC)] for h in range(2)]
            for k in range(KC):
                load(ENGS, xTh[0][k], xT_d[k * 128:(k + 1) * 128, 0:512])
            wq = {}
            for fc in (4, 5):
                for k in range(KC):
                    t = sbw.tile([128, 384], BF16, tag=f"wq{fc}_{k}", name=f"wq{fc}_{k}")
                    load(ENGS, t, wq_d[k * 128:(k + 1) * 128, fc * 384:(fc + 1) * 384])
                    wq[(fc, k)] = t
            for k in range(KC):
                load(ENGS, xTh[1][k], xT_d[k * 128:(k + 1) * 128, 512:1024])

            def xt_ap(k, n):
                return xTh[n // 4][k][:, (n % 4) * 128:(n % 4 + 1) * 128]
            for i, fc in enumerate((0, 2, 1, 3)):
                for k in range(KC):
                    t = sbw.tile([128, 390], BF16, tag=f"wq{fc}_{k}", name=f"wq{fc}_{k}")
                    load((nc.sync, nc.gpsimd), t,
                         wqx_d[k * 128:(k + 1) * 128, i * 390:(i + 1) * 390])
                    wq[(fc, k)] = t
            wp = [sba.tile([128, C], BF16, tag=f"wp{k}", name=f"wp{k}") for k in range(KC)]
            for k in range(KC):
                load((nc.sync, nc.gpsimd), wp[k], wp_d[k * 128:(k + 1) * 128, :])
            import concourse.bass as bass
            bias_sb = sba.tile([128, C], F32, tag="bias")
            bias_bcast = bass.AP(tensor=bias_d.tensor if hasattr(bias_d, 'tensor') else bias_d,
                                 offset=0, ap=[[0, 128], [1, C]])
            nc.gpsimd.dma_start(out=bias_sb, in_=bias_bcast)
            ones_sb = sba.tile([128, 16], BF16, tag="ones")
            nc.sync.dma_start(out=ones_sb, in_=ones_d[:, :])
            ident_sb = sba.tile([128, 128], BF16, tag="ident")
            nc.sync.dma_start(out=ident_sb, in_=ident_d[:, :])

            # persistent attention-side tensors.  qkT interleaves q/k head
            # pairs as 1024-col blocks [q0 k0 q1 k1 ...] so one strided copy
            # evacuates a whole norm-chunk's 6 transposes.
            qkT = sba.tile([128, 12 * N], BF16, tag="qkT")
            v1 = [sba.tile([128, H, D + 1], BF16, tag=f"v1{n}", name=f"v1{n}") for n in range(NP)]
            aoT = [sba.tile([128, N], BF16, tag=f"aoT{p}", name=f"aoT{p}") for p in range(6)]

            def m1_chunk(pool, fc, n):
                """One M1 accumulation: psum[128,384(+6)] = x-chunk @ w-cols.
                qk chunks carry 6 extra head-sum columns (wqx augmentation)."""
                w = 390 if fc < 4 else 384
                pt = pool.tile([128, 390], F32, tag="m1", name="m1ps")
                for k in range(KC):
                    nc.tensor.matmul(
                        pt[:, 0:w],
                        xt_ap(k, n),
                        wq[(fc, k)][:, :],
                        start=(k == 0), stop=(k == KC - 1))
                return pt

            def evac_v(fc, n, pt):
                g0 = (fc - 4) * G
                nc.scalar.copy(
                    v1[n][:, g0:g0 + G, 0:D],
                    pt[:, 0:384].rearrange("p (g d) -> p g d", g=G))
                if fc == 5:
                    nc.vector.tensor_copy(
                        v1[n][:, :, D:D + 1].rearrange("p h one -> p (h one)"),
                        ones_sb[:, 0:H])

            tx_count = [0]

            def evac_qk(pool, psT, fcq, fck, n):
                """Normalize 6 q-heads + 6 k-heads (same n-slice) jointly.
                Head sums arrive in psum cols 384:390 (wqx augmentation).

                Returns a closure emitting the PE transposes + evac copies,
                so the caller can software-pipeline them one iteration
                behind (keeps the in-order PE from stalling on norm)."""
                ptq = m1_chunk(pool, fcq, n)
                ptk = m1_chunk(pool, fck, n)
                G2 = 2 * G
                sq = sbt.tile([128, 768], F32, tag="sq")
                sumsq = sbt.tile([128, G2], F32, tag="sumsq")
                for i, pt in enumerate((ptq, ptk)):
                    nc.scalar.activation(out=sq[:, i * 384:(i + 1) * 384],
                                         in_=pt[:, 0:384], func=AF.Square)
                nc.vector.tensor_reduce(
                    out=sumsq, in_=sq[:, :].rearrange("p (g d) -> p g d", g=G2),
                    axis=AX.X, op=ALU.add)
                mean = sbt.tile([128, G2], F32, tag="mean")
                for i, pt in enumerate((ptq, ptk)):
                    nc.vector.tensor_scalar(out=mean[:, i * G:(i + 1) * G],
                                            in0=pt[:, 384:390], scalar1=1.0 / D,
                                            scalar2=None, op0=ALU.mult)
                var63 = sbt.tile([128, G2], F32, tag="var63")
                nc.gpsimd.tensor_tensor(out=var63, in0=mean, in1=mean, op=ALU.mult)
                nc.gpsimd.tensor_scalar(out=var63, in0=var63, scalar1=-float(D),
                                        scalar2=None, op0=ALU.mult)
                nc.gpsimd.tensor_tensor(out=var63, in0=var63, in1=sumsq, op=ALU.add)
                rstd = sbt.tile([128, G2], F32, tag="rstd")
                nc.scalar.activation(out=rstd, in_=var63, func=AF.Sqrt,
                                     scale=1.0 / (D - 1))
                nc.vector.reciprocal(out=rstd, in_=rstd)
                nmr = sbt.tile([128, G2], F32, tag="nmr")
                nc.gpsimd.tensor_tensor(out=nmr, in0=mean, in1=rstd, op=ALU.mult)
                nc.gpsimd.tensor_scalar(out=nmr, in0=nmr, scalar1=-1.0,
                                        scalar2=None, op0=ALU.mult)
                qkns = []
                for i, pt in enumerate((ptq, ptk)):
                    grp = pt[:, 0:384].rearrange("p (g d) -> p g d", g=G)
                    qkn = sbt.tile([128, 384], BF16, tag=f"qkn{i}", name=f"qkn{i}")
                    qkng = qkn[:, :].rearrange("p (g d) -> p g d", g=G)
                    for g in range(G):
                        gg = i * G + g
                        if g % 2 == 0:
                            nc.vector.tensor_scalar(
                                out=qkng[:, g, :], in0=grp[:, g, :],
                                scalar1=mean[:, gg:gg + 1], scalar2=rstd[:, gg:gg + 1],
                                op0=ALU.subtract, op1=ALU.mult)
                        else:
                            nc.scalar.activation(out=qkng[:, g, :], in_=grp[:, g, :],
                                                 func=AF.Identity, bias=nmr[:, gg:gg + 1],
                                                 scale=rstd[:, gg:gg + 1])
                    qkns.append(qkn)

                def emit_transposes():
                    bp2 = 0 if fcq in (0, 2) else 6
                    ptt = psT.tile([128, 768], BF16, tag="tp", name="tpps")
                    for j in range(3):
                        nc.tensor.transpose(
                            ptt[:, (2 * j) * 128:(2 * j + 1) * 128],
                            qkns[0][:, j * 128:(j + 1) * 128], ident_sb[:, :])
                        nc.tensor.transpose(
                            ptt[:, (2 * j + 1) * 128:(2 * j + 2) * 128],
                            qkns[1][:, j * 128:(j + 1) * 128], ident_sb[:, :])
                    dst = qkT[:, :].rearrange(
                        "p (b n c) -> p b n c", b=12, n=NP)[:, bp2:bp2 + 6, n, :]
                    src = ptt[:, :].rearrange("p (j c) -> p j c", j=6)
                    if tx_count[0] % 2 == 0:
                        nc.vector.tensor_copy(dst, src)
                    else:
                        nc.scalar.copy(dst, src)
                    tx_count[0] += 1
                return emit_transposes

            def attention_pair(p, sbe, ps2, ps3):
                for par in range(2):
                    h = 2 * p + par
                    et = [sbe.tile([128, N], BF16, tag=f"e{par}{nk}", name=f"e{par}{nk}") for nk in range(NP)]
                    qb, kb = (2 * p) * N, (2 * p + 1) * N
                    for nk in range(NP):
                        p2 = ps2.tile([128, 1024], F32, tag="m2", name="m2ps")
                        for nqh in range(2):
                            nc.tensor.matmul(
                                p2[:, nqh * 512:(nqh + 1) * 512],
                                qkT[par * D:(par + 1) * D, kb + nk * 128:kb + (nk + 1) * 128],
                                qkT[par * D:(par + 1) * D, qb + nqh * 512:qb + (nqh + 1) * 512],
                                start=True, stop=True)
                        nc.scalar.activation(
                            out=et[nk][:, :],
                            in_=p2[:, :], func=AF.Exp, scale=float(D) ** -0.5)
                    for nqh in range(2):
                        p3 = ps3.tile([D + 1, 512], F32, tag="m3", name="m3ps")
                        for nk in range(NP):
                            nc.tensor.matmul(
                                p3[:, :],
                                v1[nk][:, h, :],
                                et[nk][:, nqh * 512:(nqh + 1) * 512],
                                start=(nk == 0), stop=(nk == NP - 1))
                        sS = sbt.tile([1, 512], F32, tag="sS")
                        nc.vector.tensor_copy(sS[:, :], p3[D:D + 1, :])
                        rS = sbt.tile([1, 512], F32, tag="rS")
                        nc.vector.reciprocal_approx_fast(out=rS, in_=sS[:, :])
                        rSb = sbt.tile([D, 512], F32, tag="rSb")
                        nc.gpsimd.partition_broadcast(rSb[:, :], rS[:, :])
                        nc.vector.tensor_tensor(
                            out=aoT[p][par * D:(par + 1) * D,
                                       nqh * 512:(nqh + 1) * 512],
                            in0=p3[0:D, :], in1=rSb[:, :], op=ALU.mult)

            # ---------- phase 1: M1 + norm ----------
            ps1_cm = tc.tile_pool(name="ps1", bufs=6, space="PSUM")
            ps1 = ps1_cm.__enter__()
            psT_cm = tc.tile_pool(name="psT", bufs=2, space="PSUM")
            psT = psT_cm.__enter__()
            for fc in (4, 5):                      # v first
                for n in range(NP):
                    evac_v(fc, n, m1_chunk(ps1, fc, n))
            pending = None
            for fcq, fck in ((0, 2), (1, 3)):      # q,k (sqrt-table phase)
                for n in range(NP):
                    w = evac_qk(ps1, psT, fcq, fck, n)
                    if pending is not None:
                        pending()
                    pending = w
            pending()
            psT_cm.__exit__(None, None, None)
            ps1_cm.__exit__(None, None, None)
            sbw_cm.__exit__(None, None, None)      # free x/w region for expT

            # ---------- phase 2: attention (exp-table phase) ----------
            sbe_cm = tc.tile_pool(name="sbe", bufs=2)
            sbe = sbe_cm.__enter__()
            ps2_cm = tc.tile_pool(name="ps2", bufs=2, space="PSUM")
            ps2 = ps2_cm.__enter__()
            ps3_cm = tc.tile_pool(name="ps3", bufs=4, space="PSUM")
            ps3 = ps3_cm.__enter__()
            for p in range(6):
                attention_pair(p, sbe, ps2, ps3)
            ps3_cm.__exit__(None, None, None)
            ps2_cm.__exit__(None, None, None)
            sbe_cm.__exit__(None, None, None)

            # ---------- phase 3: proj ----------
            psP_cm = tc.tile_pool(name="psP", bufs=4, space="PSUM")
            psP = psP_cm.__enter__()
            for n in range(NP):
                ysb = sbt.tile([128, C], F32, tag="y")
                for half in range(2):
                    pp = psP.tile([128, 384], F32, tag="mp", name="mpps")
                    for k in range(KC):
                        nc.tensor.matmul(
                            pp[:, :],
                            aoT[k][:, n * 128:(n + 1) * 128],
                            wp[k][:, half * 384:(half + 1) * 384],
                            start=(k == 0), stop=(k == KC - 1))
                    nc.vector.tensor_tensor(
                        out=ysb[:, half * 384:(half + 1) * 384], in0=pp[:, :],
                        in1=bias_sb[:, half * 384:(half + 1) * 384], op=ALU.add)
                    nc.sync.dma_start(
                        out=y_d[n * 128:(n + 1) * 128, half * 384:(half + 1) * 384],
                        in_=ysb[:, half * 384:(half + 1) * 384])
            psP_cm.__exit__(None, None, None)

    nc.compile()
    return nc


def _prep_inputs(x, qkv_w, proj_w, proj_b):
    import ml_dtypes
    wqkvT = np.ascontiguousarray(qkv_w.T).astype(ml_dtypes.bfloat16)    # [768, 2304]
    wpT = np.ascontiguousarray(proj_w.T).astype(ml_dtypes.bfloat16)     # [768, 768]
    bias = proj_b.reshape(1, C).astype(np.float32)
    ones16 = np.ones((128, 16), dtype=ml_dtypes.bfloat16)
    ident = np.eye(128, dtype=ml_dtypes.bfloat16)
    # q/k weight chunks augmented with per-head column sums (fc order 0,2,1,3)
    wqx = np.zeros((C, 4, 390), dtype=np.float64)
    for i, fc in enumerate((0, 2, 1, 3)):
        cols = qkv_w.T[:, fc * 384:(fc + 1) * 384].astype(np.float64)
        wqx[:, i, 0:384] = cols
        wqx[:, i, 384:390] = cols.reshape(C, 6, D).sum(-1)
    wqxT = np.ascontiguousarray(wqx.reshape(C, 4 * 390)).astype(ml_dtypes.bfloat16)
    maps = []
    for b in range(B):
        maps.append({
            "xT": np.ascontiguousarray(x[b].T).astype(ml_dtypes.bfloat16),
            "wqkvT": wqkvT, "wpT": wpT, "bias": bias, "ones16": ones16,
            "ident": ident, "wqxT": wqxT,
        })
    return maps


def kernel(x, qkv_w, proj_w, proj_b):
    from concourse.bass_utils import run_bass_kernel_spmd
    if "nc" not in _CACHE:
        _CACHE["nc"] = _build_nc()
    nc = _CACHE["nc"]
    maps = _prep_inputs(np.asarray(x), np.asarray(qkv_w),
                        np.asarray(proj_w), np.asarray(proj_b))
    import os
    trace = bool(os.environ.get("KERNEL_TRACE"))
    res = run_bass_kernel_spmd(nc, maps, list(range(B)), trace=trace)
    _CACHE["last_result"] = res
    out = np.stack([res.results[b]["y"] for b in range(B)], axis=0)
    return out.astype(np.float32)


# revision 48
# speedup vs baseline: 1.3693x; 1.0353x over previous
"""Trainium2 Bass kernel for nn_Attention_74088185856351.

Strategy: data-parallel over batch (8 batches -> 8 NeuronCores), full
attention per core. Host pre-transposes x / weights so the device does
only matmuls, norm stats, exp.

Per-core pipeline (N=1024 tokens, C=768, H=12 heads, D=64):
  M1   : qkv = xT.T @ qkv_w.T     (bf16 matmuls, fp32 PSUM accum)
  norm : per-head standardization of q,k (ddof=1), fused with PSUM
         evacuation; outputs bf16
  qT/kT: DMA-transpose (bf16) to [d, n] layout
  M2   : logitsT[nk,nq] per head  (bf16, K=64)
  exp  : ACT Exp with 1/8 scale folded, PSUM->SBUF bf16
  M3   : outT'[65,nq] = [v|1].T @ expT  -> row 64 = softmax denom S
  evac : out = outT/S  (fast-approx reciprocal + gpsimd
         partition_broadcast + mult)
  proj : y = attnoutT.T @ proj_w.T + bias (bf16 matmul, fp32 out)
"""
import sys
sys.path.insert(0, '/opt/trn_rl_repo')
import numpy as np

B, N, C, H = 8, 1024, 768, 12
D = C // H          # 64
NP = N // 128       # 8 n-chunks
KC = C // 128       # 6 contraction chunks
G = 6               # heads per 384-wide M1 chunk

_CACHE = {}


def _build_nc():
    import concourse.bacc as bacc
    import concourse.tile as tile
    from concourse import mybir

    F32, BF16 = mybir.dt.float32, mybir.dt.bfloat16
    AX, ALU, AF = mybir.AxisListType, mybir.AluOpType, mybir.ActivationFunctionType

    nc = bacc.Bacc("TRN2", target_bir_lowering=False, debug=False, num_devices=8)
    xT_d = nc.declare_dram_parameter("xT", [C, N], BF16, isOutput=False)
    wq_d = nc.declare_dram_parameter("wqkvT", [C, 3 * C], BF16, isOutput=False)
    wp_d = nc.declare_dram_parameter("wpT", [C, C], BF16, isOutput=False)
    bias_d = nc.declare_dram_parameter("bias", [1, C], F32, isOutput=False)
    ones_d = nc.declare_dram_parameter("ones16", [128, 16], BF16, isOutput=False)
    ident_d = nc.declare_dram_parameter("ident", [128, 128], BF16, isOutput=False)
    # q/k weight chunks augmented with 6 per-head column-sum columns
    # ([C, 4, 390], fc order 0,2,1,3): head-sums fall out of the same M1
    # matmul at F-cost +6 instead of a DVE reduction per chunk
    wqx_d = nc.declare_dram_parameter("wqxT", [C, 4 * 390], BF16, isOutput=False)
    y_d = nc.declare_dram_parameter("y", [N, C], F32, isOutput=True)

    # M1 column-chunk consumption order: v chunks first, then q/k pairs
    FC_ORDER = (4, 5, 0, 2, 1, 3)

    with tile.TileContext(nc) as tc:
        with (
            tc.tile_pool(name="sba", bufs=1) as sba,
            tc.tile_pool(name="sbt", bufs=5) as sbt,
        ):
            sbw_cm = tc.tile_pool(name="sbw", bufs=1)
            sbw = sbw_cm.__enter__()
            # ---- loads (in consumption order; v-phase weights on the
            # scalar queue so they land while xT streams on sync) ----
            # critical first loads (xT + v-phase weights) round-robin all
            # 3 DMA-capable queues; later loads stay off the scalar queue
            # so ACT compute is never stuck behind descriptor generation.
            rr = [0]
            ENGS = (nc.sync, nc.scalar, nc.gpsimd)

            def load(eng_set, out, in_):
                eng_set[rr[0] % len(eng_set)].dma_start(out=out, in_=in_)
                rr[0] += 1

            # xT as separate half tiles: n-chunks 0-3 in xTh[0][k], 4-7 in
            # xTh[1][k] — lets the v-phase start on ~1.9MB instead of 8.5MB
            xTh = [[sbw.tile([128, 512], BF16, tag=f"xT{h}_{k}", name=f"xT{h}_{k}")
                    for k in range(KC)] for h in range(2)]
            for k in range(KC):
                load(ENGS, xTh[0][k], xT_d[k * 128:(k + 1) * 128, 0:512])
            wq = {}

            def load_wq(fc):
                qk_idx = {0: 0, 2: 1, 1: 2, 3: 3}
                for k in range(KC):
                    if fc in (4, 5):
                        t = sbw.tile([128, 384], BF16, tag=f"wq{fc}_{k}", name=f"wq{fc}_{k}")
                        load(ENGS, t, wq_d[k * 128:(k + 1) * 128, fc * 384:(fc + 1) * 384])
                    else:
                        i = qk_idx[fc]
                        t = sbw.tile([128, 390], BF16, tag=f"wq{fc}_{k}", name=f"wq{fc}_{k}")
                        load((nc.sync, nc.gpsimd), t,
                             wqx_d[k * 128:(k + 1) * 128, i * 390:(i + 1) * 390])
                    wq[(fc, k)] = t

            load_wq(4)
            for k in range(KC):
                load(ENGS, xTh[1][k], xT_d[k * 128:(k + 1) * 128, 512:1024])

            def xt_ap(k, n):
                return xTh[n // 4][k][:, (n % 4) * 128:(n % 4 + 1) * 128]
            for fc in (0, 2, 5, 1, 3):
                load_wq(fc)
            wp = [sba.tile([128, C], BF16, tag=f"wp{k}", name=f"wp{k}") for k in range(KC)]
            for k in range(KC):
                load((nc.sync, nc.gpsimd), wp[k], wp_d[k * 128:(k + 1) * 128, :])
            import concourse.bass as bass
            bias_sb = sba.tile([128, C], F32, tag="bias")
            bias_bcast = bass.AP(tensor=bias_d.tensor if hasattr(bias_d, 'tensor') else bias_d,
                                 offset=0, ap=[[0, 128], [1, C]])
            nc.gpsimd.dma_start(out=bias_sb, in_=bias_bcast)
            ones_sb = sba.tile([128, 16], BF16, tag="ones")
            nc.sync.dma_start(out=ones_sb, in_=ones_d[:, :])
            ident_sb = sba.tile([128, 128], BF16, tag="ident")
            nc.sync.dma_start(out=ident_sb, in_=ident_d[:, :])

            # persistent attention-side tensors.  qkT interleaves q/k head
            # pairs as 1024-col blocks [q0 k0 q1 k1 ...] so one strided copy
            # evacuates a whole norm-chunk's 6 transposes.
            qkT = sba.tile([128, 12 * N], BF16, tag="qkT")
            v1 = [sba.tile([128, H, D + 1], BF16, tag=f"v1{n}", name=f"v1{n}") for n in range(NP)]
            aoT = [sba.tile([128, N], BF16, tag=f"aoT{p}", name=f"aoT{p}") for p in range(6)]

            def m1_chunk(pool, fc, n):
                """One M1 accumulation: psum[128,384(+6)] = x-chunk @ w-cols.
                qk chunks carry 6 extra head-sum columns (wqx augmentation)."""
                w = 390 if fc < 4 else 384
                pt = pool.tile([128, 390], F32, tag="m1", name="m1ps")
                for k in range(KC):
                    nc.tensor.matmul(
                        pt[:, 0:w],
                        xt_ap(k, n),
                        wq[(fc, k)][:, :],
                        start=(k == 0), stop=(k == KC - 1))
                return pt

            def evac_v(fc, n, pt):
                g0 = (fc - 4) * G
                nc.scalar.copy(
                    v1[n][:, g0:g0 + G, 0:D],
                    pt[:, 0:384].rearrange("p (g d) -> p g d", g=G))
                if fc == 5:
                    nc.vector.tensor_copy(
                        v1[n][:, :, D:D + 1].rearrange("p h one -> p (h one)"),
                        ones_sb[:, 0:H])

            tx_count = [0]

            def evac_qk(pool, psT, fcq, fck, n):
                """Normalize 6 q-heads + 6 k-heads (same n-slice) jointly.
                Head sums arrive in psum cols 384:390 (wqx augmentation).

                Returns a closure emitting the PE transposes + evac copies,
                so the caller can software-pipeline them one iteration
                behind (keeps the in-order PE from stalling on norm)."""
                ptq = m1_chunk(pool, fcq, n)
                ptk = m1_chunk(pool, fck, n)
                G2 = 2 * G
                sq = sbt.tile([128, 768], F32, tag="sq")
                sumsq = sbt.tile([128, G2], F32, tag="sumsq")
                for i, pt in enumerate((ptq, ptk)):
                    nc.scalar.activation(out=sq[:, i * 384:(i + 1) * 384],
                                         in_=pt[:, 0:384], func=AF.Square)
                nc.vector.tensor_reduce(
                    out=sumsq, in_=sq[:, :].rearrange("p (g d) -> p g d", g=G2),
                    axis=AX.X, op=ALU.add)
                mean = sbt.tile([128, G2], F32, tag="mean")
                for i, pt in enumerate((ptq, ptk)):
                    nc.vector.tensor_scalar(out=mean[:, i * G:(i + 1) * G],
                                            in0=pt[:, 384:390], scalar1=1.0 / D,
                                            scalar2=None, op0=ALU.mult)
                var63 = sbt.tile([128, G2], F32, tag="var63")
                nc.gpsimd.tensor_tensor(out=var63, in0=mean, in1=mean, op=ALU.mult)
                nc.gpsimd.tensor_scalar(out=var63, in0=var63, scalar1=-float(D),
                                        scalar2=None, op0=ALU.mult)
                nc.gpsimd.tensor_tensor(out=var63, in0=var63, in1=sumsq, op=ALU.add)
                rstd = sbt.tile([128, G2], F32, tag="rstd")
                nc.scalar.activation(out=rstd, in_=var63, func=AF.Sqrt,
                                     scale=1.0 / (D - 1))
                nc.vector.reciprocal(out=rstd, in_=rstd)
                nmr = sbt.tile([128, G2], F32, tag="nmr")
                nc.gpsimd.tensor_tensor(out=nmr, in0=mean, in1=rstd, op=ALU.mult)
                nc.gpsimd.tensor_scalar(out=nmr, in0=nmr, scalar1=-1.0,
                                        scalar2=None, op0=ALU.mult)
                qkns = []
                for i, pt in enumerate((ptq, ptk)):
                    grp = pt[:, 0:384].rearrange("p (g d) -> p g d", g=G)
                    qkn = sbt.tile([128, 384], BF16, tag=f"qkn{i}", name=f"qkn{i}")
                    qkng = qkn[:, :].rearrange("p (g d) -> p g d", g=G)
                    for g in range(G):
                        gg = i * G + g
                        if g % 2 == 0:
                            nc.vector.tensor_scalar(
                                out=qkng[:, g, :], in0=grp[:, g, :],
                                scalar1=mean[:, gg:gg + 1], scalar2=rstd[:, gg:gg + 1],
                                op0=ALU.subtract, op1=ALU.mult)
                        else:
                            nc.scalar.activation(out=qkng[:, g, :], in_=grp[:, g, :],
                                                 func=AF.Identity, bias=nmr[:, gg:gg + 1],
                                                 scale=rstd[:, gg:gg + 1])
                    qkns.append(qkn)

                def emit_transposes():
                    bp2 = 0 if fcq in (0, 2) else 6
                    ptt = psT.tile([128, 768], BF16, tag="tp", name="tpps")
                    for j in range(3):
                        nc.tensor.transpose(
                            ptt[:, (2 * j) * 128:(2 * j + 1) * 128],
                            qkns[0][:, j * 128:(j + 1) * 128], ident_sb[:, :])
                        nc.tensor.transpose(
                            ptt[:, (2 * j + 1) * 128:(2 * j + 2) * 128],
                            qkns[1][:, j * 128:(j + 1) * 128], ident_sb[:, :])
                    dst = qkT[:, :].rearrange(
                        "p (b n c) -> p b n c", b=12, n=NP)[:, bp2:bp2 + 6, n, :]
                    src = ptt[:, :].rearrange("p (j c) -> p j c", j=6)
                    if tx_count[0] % 2 == 0:
                        nc.vector.tensor_copy(dst, src)
                    else:
                        nc.scalar.copy(dst, src)
                    tx_count[0] += 1
                return emit_transposes

            def attention_pair(p, sbe, ps2, ps3):
                for par in range(2):
                    h = 2 * p + par
                    et = [sbe.tile([128, N], BF16, tag=f"e{par}{nk}", name=f"e{par}{nk}") for nk in range(NP)]
                    qb, kb = (2 * p) * N, (2 * p + 1) * N
                    for nk in range(NP):
                        p2 = ps2.tile([128, 1024], F32, tag="m2", name="m2ps")
                        for nqh in range(2):
                            nc.tensor.matmul(
                                p2[:, nqh * 512:(nqh + 1) * 512],
                                qkT[par * D:(par + 1) * D, kb + nk * 128:kb + (nk + 1) * 128],
                                qkT[par * D:(par + 1) * D, qb + nqh * 512:qb + (nqh + 1) * 512],
                                start=True, stop=True)
                        nc.scalar.activation(
                            out=et[nk][:, :],
                            in_=p2[:, :], func=AF.Exp, scale=float(D) ** -0.5)
                    for nqh in range(2):
                        p3 = ps3.tile([D + 1, 512], F32, tag="m3", name="m3ps")
                        for nk in range(NP):
                            nc.tensor.matmul(
                                p3[:, :],
                                v1[nk][:, h, :],
                                et[nk][:, nqh * 512:(nqh + 1) * 512],
                                start=(nk == 0), stop=(nk == NP - 1))
                        sS = sbt.tile([1, 512], F32, tag="sS")
                        nc.vector.tensor_copy(sS[:, :], p3[D:D + 1, :])
                        rS = sbt.tile([1, 512], F32, tag="rS")
                        nc.vector.reciprocal_approx_fast(out=rS, in_=sS[:, :])
                        rSb = sbt.tile([D, 512], F32, tag="rSb")
                        nc.gpsimd.partition_broadcast(rSb[:, :], rS[:, :])
                        nc.vector.tensor_tensor(
                            out=aoT[p][par * D:(par + 1) * D,
                                       nqh * 512:(nqh + 1) * 512],
                            in0=p3[0:D, :], in1=rSb[:, :], op=ALU.mult)

            # ---------- phase 1: M1 + norm ----------
            ps1_cm = tc.tile_pool(name="ps1", bufs=6, space="PSUM")
            ps1 = ps1_cm.__enter__()
            psT_cm = tc.tile_pool(name="psT", bufs=2, space="PSUM")
            psT = psT_cm.__enter__()
            # v chunks interleave into the qk passes: v matmuls are PE-bound
            # with idle DVE/ACT, qk norm is DVE/ACT-bound with PE slack
            pending = None
            for fcv, (fcq, fck) in ((4, (0, 2)), (5, (1, 3))):
                for n in range(NP):
                    evac_v(fcv, n, m1_chunk(ps1, fcv, n))
                    w = evac_qk(ps1, psT, fcq, fck, n)
                    if pending is not None:
                        pending()
                    pending = w
            pending()
            psT_cm.__exit__(None, None, None)
            ps1_cm.__exit__(None, None, None)
            sbw_cm.__exit__(None, None, None)      # free x/w region for expT

            # ---------- phase 2: attention (exp-table phase) ----------
            sbe_cm = tc.tile_pool(name="sbe", bufs=2)
            sbe = sbe_cm.__enter__()
            ps2_cm = tc.tile_pool(name="ps2", bufs=2, space="PSUM")
            ps2 = ps2_cm.__enter__()
            ps3_cm = tc.tile_pool(name="ps3", bufs=4, space="PSUM")
            ps3 = ps3_cm.__enter__()
            for p in range(6):
                attention_pair(p, sbe, ps2, ps3)
            ps3_cm.__exit__(None, None, None)
            ps2_cm.__exit__(None, None, None)
            sbe_cm.__exit__(None, None, None)

            # ---------- phase 3: proj ----------
            psP_cm = tc.tile_pool(name="psP", bufs=4, space="PSUM")
            psP = psP_cm.__enter__()
            for n in range(NP):
                ysb = sbt.tile([128, C], F32, tag="y")
                for half in range(2):
                    pp = psP.tile([128, 384], F32, tag="mp", name="mpps")
                    for k in range(KC):
                        nc.tensor.matmul(
                            pp[:, :],
                            aoT[k][:, n * 128:(n + 1) * 128],
                            wp[k][:, half * 384:(half + 1) * 384],
                            start=(k == 0), stop=(k == KC - 1))
                    nc.vector.tensor_tensor(
                        out=ysb[:, half * 384:(half + 1) * 384], in0=pp[:, :],
                        in1=bias_sb[:, half * 384:(half + 1) * 384], op=ALU.add)
                    nc.sync.dma_start(
                        out=y_d[n * 128:(n + 1) * 128, half * 384:(half + 1) * 384],
                        in_=ysb[:, half * 384:(half + 1) * 384])
            psP_cm.__exit__(None, None, None)

    nc.compile()
    return nc


def _prep_inputs(x, qkv_w, proj_w, proj_b):
    import ml_dtypes
    wqkvT = np.ascontiguousarray(qkv_w.T).astype(ml_dtypes.bfloat16)    # [768, 2304]
    wpT = np.ascontiguousarray(proj_w.T).astype(ml_dtypes.bfloat16)     # [768, 768]
    bias = proj_b.reshape(1, C).astype(np.float32)
    ones16 = np.ones((128, 16), dtype=ml_dtypes.bfloat16)
    ident = np.eye(128, dtype=ml_dtypes.bfloat16)
    # q/k weight chunks augmented with per-head column sums (fc order 0,2,1,3)
    wqx = np.zeros((C, 4, 390), dtype=np.float64)
    for i, fc in enumerate((0, 2, 1, 3)):
        cols = qkv_w.T[:, fc * 384:(fc + 1) * 384].astype(np.float64)
        wqx[:, i, 0:384] = cols
        wqx[:, i, 384:390] = cols.reshape(C, 6, D).sum(-1)
    wqxT = np.ascontiguousarray(wqx.reshape(C, 4 * 390)).astype(ml_dtypes.bfloat16)
    maps = []
    for b in range(B):
        maps.append({
            "xT": np.ascontiguousarray(x[b].T).astype(ml_dtypes.bfloat16),
            "wqkvT": wqkvT, "wpT": wpT, "bias": bias, "ones16": ones16,
            "ident": ident, "wqxT": wqxT,
        })
    return maps


def kernel(x, qkv_w, proj_w, proj_b):
    from concourse.bass_utils import run_bass_kernel_spmd
    if "nc" not in _CACHE:
        _CACHE["nc"] = _build_nc()
    nc = _CACHE["nc"]
    maps = _prep_inputs(np.asarray(x), np.asarray(qkv_w),
                        np.asarray(proj_w), np.asarray(proj_b))
    import os
    trace = bool(os.environ.get("KERNEL_TRACE"))
    res = run_bass_kernel_spmd(nc, maps, list(range(B)), trace=trace)
    _CACHE["last_result"] = res
    out = np.stack([res.results[b]["y"] for b in range(B)], axis=0)
    return out.astype(np.float32)


# revision 49
# speedup vs baseline: 1.4231x; 1.0393x over previous
"""Trainium2 Bass kernel for nn_Attention_74088185856351.

Strategy: data-parallel over batch (8 batches -> 8 NeuronCores), full
attention per core. Host pre-transposes x / weights so the device does
only matmuls, norm stats, exp.

Per-core pipeline (N=1024 tokens, C=768, H=12 heads, D=64):
  M1   : qkv = xT.T @ qkv_w.T     (bf16 matmuls, fp32 PSUM accum)
  norm : per-head standardization of q,k (ddof=1), fused with PSUM
         evacuation; outputs bf16
  qT/kT: DMA-transpose (bf16) to [d, n] layout
  M2   : logitsT[nk,nq] per head  (bf16, K=64)
  exp  : ACT Exp with 1/8 scale folded, PSUM->SBUF bf16
  M3   : outT'[65,nq] = [v|1].T @ expT  -> row 64 = softmax denom S
  evac : out = outT/S  (fast-approx reciprocal + gpsimd
         partition_broadcast + mult)
  proj : y = attnoutT.T @ proj_w.T + bias (bf16 matmul, fp32 out)
"""
import sys
sys.path.insert(0, '/opt/trn_rl_repo')
import numpy as np

B, N, C, H = 8, 1024, 768, 12
D = C // H          # 64
NP = N // 128       # 8 n-chunks
KC = C // 128       # 6 contraction chunks
G = 6               # heads per 384-wide M1 chunk

_CACHE = {}


def _build_nc():
    import concourse.bacc as bacc
    import concourse.tile as tile
    from concourse import mybir

    F32, BF16 = mybir.dt.float32, mybir.dt.bfloat16
    AX, ALU, AF = mybir.AxisListType, mybir.AluOpType, mybir.ActivationFunctionType

    nc = bacc.Bacc("TRN2", target_bir_lowering=False, debug=False, num_devices=8)
    xT_d = nc.declare_dram_parameter("xT", [C, N], BF16, isOutput=False)
    wq_d = nc.declare_dram_parameter("wqkvT", [C, 3 * C], BF16, isOutput=False)
    wp_d = nc.declare_dram_parameter("wpT", [C, C], BF16, isOutput=False)
    bias_d = nc.declare_dram_parameter("bias", [1, C], F32, isOutput=False)
    ones_d = nc.declare_dram_parameter("ones16", [128, 16], BF16, isOutput=False)
    ident_d = nc.declare_dram_parameter("ident", [128, 128], BF16, isOutput=False)
    # q/k weight chunks augmented with 6 per-head column-sum columns
    # ([C, 4, 390], fc order 0,2,1,3): head-sums fall out of the same M1
    # matmul at F-cost +6 instead of a DVE reduction per chunk
    wqx_d = nc.declare_dram_parameter("wqxT", [C, 4 * 390], BF16, isOutput=False)
    y_d = nc.declare_dram_parameter("y", [N, C], F32, isOutput=True)

    # M1 column-chunk consumption order: v chunks first, then q/k pairs
    FC_ORDER = (4, 5, 0, 2, 1, 3)

    with tile.TileContext(nc) as tc:
        with (
            tc.tile_pool(name="sba", bufs=1) as sba,
            tc.tile_pool(name="sbt", bufs=5) as sbt,
        ):
            sbw_cm = tc.tile_pool(name="sbw", bufs=1)
            sbw = sbw_cm.__enter__()
            # ---- loads (in consumption order; v-phase weights on the
            # scalar queue so they land while xT streams on sync) ----
            # critical first loads (xT + v-phase weights) round-robin all
            # 3 DMA-capable queues; later loads stay off the scalar queue
            # so ACT compute is never stuck behind descriptor generation.
            rr = [0]
            ENGS = (nc.sync, nc.scalar, nc.gpsimd)

            def load(eng_set, out, in_):
                eng_set[rr[0] % len(eng_set)].dma_start(out=out, in_=in_)
                rr[0] += 1

            # xT as separate half tiles: n-chunks 0-3 in xTh[0][k], 4-7 in
            # xTh[1][k] — lets the v-phase start on ~1.9MB instead of 8.5MB
            xTh = [[sbw.tile([128, 512], BF16, tag=f"xT{h}_{k}", name=f"xT{h}_{k}")
                    for k in range(KC)] for h in range(2)]
            for k in range(KC):
                load(ENGS, xTh[0][k], xT_d[k * 128:(k + 1) * 128, 0:512])
            wq = {}

            def load_wq(fc):
                qk_idx = {0: 0, 2: 1, 1: 2, 3: 3}
                for k in range(KC):
                    if fc in (4, 5):
                        t = sbw.tile([128, 384], BF16, tag=f"wq{fc}_{k}", name=f"wq{fc}_{k}")
                        load(ENGS, t, wq_d[k * 128:(k + 1) * 128, fc * 384:(fc + 1) * 384])
                    else:
                        i = qk_idx[fc]
                        t = sbw.tile([128, 390], BF16, tag=f"wq{fc}_{k}", name=f"wq{fc}_{k}")
                        load((nc.sync, nc.gpsimd), t,
                             wqx_d[k * 128:(k + 1) * 128, i * 390:(i + 1) * 390])
                    wq[(fc, k)] = t

            load_wq(4)
            for k in range(KC):
                load(ENGS, xTh[1][k], xT_d[k * 128:(k + 1) * 128, 512:1024])

            def xt_ap(k, n):
                return xTh[n // 4][k][:, (n % 4) * 128:(n % 4 + 1) * 128]
            for fc in (0, 2, 5, 1, 3):
                load_wq(fc)
            wp = [sba.tile([128, C], BF16, tag=f"wp{k}", name=f"wp{k}") for k in range(KC)]
            for k in range(KC):
                load((nc.sync, nc.gpsimd), wp[k], wp_d[k * 128:(k + 1) * 128, :])
            import concourse.bass as bass
            bias_sb = sba.tile([128, C], F32, tag="bias")
            bias_bcast = bass.AP(tensor=bias_d.tensor if hasattr(bias_d, 'tensor') else bias_d,
                                 offset=0, ap=[[0, 128], [1, C]])
            nc.gpsimd.dma_start(out=bias_sb, in_=bias_bcast)
            ones_sb = sba.tile([128, 16], BF16, tag="ones")
            nc.sync.dma_start(out=ones_sb, in_=ones_d[:, :])
            ident_sb = sba.tile([128, 128], BF16, tag="ident")
            nc.sync.dma_start(out=ident_sb, in_=ident_d[:, :])

            # persistent attention-side tensors.  qkT interleaves q/k head
            # pairs as 1024-col blocks [q0 k0 q1 k1 ...] so one strided copy
            # evacuates a whole norm-chunk's 6 transposes.
            qkT = sba.tile([128, 12 * N], BF16, tag="qkT")
            v1 = [sba.tile([128, H, D + 1], BF16, tag=f"v1{n}", name=f"v1{n}") for n in range(NP)]
            aoT = [sba.tile([128, N], BF16, tag=f"aoT{p}", name=f"aoT{p}") for p in range(6)]

            def m1_chunk(pool, fc, n):
                """One M1 accumulation: psum[128,384(+6)] = x-chunk @ w-cols.
                qk chunks carry 6 extra head-sum columns (wqx augmentation)."""
                w = 390 if fc < 4 else 384
                pt = pool.tile([128, 390], F32, tag="m1", name="m1ps")
                for k in range(KC):
                    nc.tensor.matmul(
                        pt[:, 0:w],
                        xt_ap(k, n),
                        wq[(fc, k)][:, :],
                        start=(k == 0), stop=(k == KC - 1))
                return pt

            def evac_v(fc, n, pt):
                g0 = (fc - 4) * G
                nc.scalar.copy(
                    v1[n][:, g0:g0 + G, 0:D],
                    pt[:, 0:384].rearrange("p (g d) -> p g d", g=G))
                if fc == 5:
                    nc.vector.tensor_copy(
                        v1[n][:, :, D:D + 1].rearrange("p h one -> p (h one)"),
                        ones_sb[:, 0:H])

            tx_count = [0]

            def evac_qk(pool, psT, fcq, fck, n):
                """Normalize 6 q-heads + 6 k-heads (same n-slice) jointly.
                Head sums arrive in psum cols 384:390 (wqx augmentation).

                Returns a closure emitting the PE transposes + evac copies,
                so the caller can software-pipeline them one iteration
                behind (keeps the in-order PE from stalling on norm)."""
                ptq = m1_chunk(pool, fcq, n)
                ptk = m1_chunk(pool, fck, n)
                G2 = 2 * G
                sq = sbt.tile([128, 768], F32, tag="sq")
                sumsq = sbt.tile([128, G2], F32, tag="sumsq")
                for i, pt in enumerate((ptq, ptk)):
                    nc.scalar.activation(out=sq[:, i * 384:(i + 1) * 384],
                                         in_=pt[:, 0:384], func=AF.Square)
                nc.vector.tensor_reduce(
                    out=sumsq, in_=sq[:, :].rearrange("p (g d) -> p g d", g=G2),
                    axis=AX.X, op=ALU.add)
                mean = sbt.tile([128, G2], F32, tag="mean")
                for i, pt in enumerate((ptq, ptk)):
                    nc.vector.tensor_scalar(out=mean[:, i * G:(i + 1) * G],
                                            in0=pt[:, 384:390], scalar1=1.0 / D,
                                            scalar2=None, op0=ALU.mult)
                var63 = sbt.tile([128, G2], F32, tag="var63")
                nc.gpsimd.tensor_tensor(out=var63, in0=mean, in1=mean, op=ALU.mult)
                nc.gpsimd.tensor_scalar(out=var63, in0=var63, scalar1=-float(D),
                                        scalar2=None, op0=ALU.mult)
                nc.gpsimd.tensor_tensor(out=var63, in0=var63, in1=sumsq, op=ALU.add)
                rstd = sbt.tile([128, G2], F32, tag="rstd")
                nc.scalar.activation(out=rstd, in_=var63, func=AF.Sqrt,
                                     scale=1.0 / (D - 1))
                nc.vector.reciprocal(out=rstd, in_=rstd)
                nmr = sbt.tile([128, G2], F32, tag="nmr")
                nc.gpsimd.tensor_tensor(out=nmr, in0=mean, in1=rstd, op=ALU.mult)
                nc.gpsimd.tensor_scalar(out=nmr, in0=nmr, scalar1=-1.0,
                                        scalar2=None, op0=ALU.mult)
                qkns = []
                for i, pt in enumerate((ptq, ptk)):
                    grp = pt[:, 0:384].rearrange("p (g d) -> p g d", g=G)
                    qkn = sbt.tile([128, 384], BF16, tag=f"qkn{i}", name=f"qkn{i}")
                    qkng = qkn[:, :].rearrange("p (g d) -> p g d", g=G)
                    rb = rstd[:, i * G:(i + 1) * G].broadcast_to((128, G, D))
                    ab = nmr[:, i * G:(i + 1) * G].broadcast_to((128, G, D))
                    nc.vector.tensor_tensor(out=qkng, in0=grp, in1=rb, op=ALU.mult)
                    nc.vector.tensor_tensor(out=qkng, in0=qkng, in1=ab, op=ALU.add)
                    qkns.append(qkn)

                def emit_transposes():
                    bp2 = 0 if fcq in (0, 2) else 6
                    ptt = psT.tile([128, 768], BF16, tag="tp", name="tpps")
                    for j in range(3):
                        nc.tensor.transpose(
                            ptt[:, (2 * j) * 128:(2 * j + 1) * 128],
                            qkns[0][:, j * 128:(j + 1) * 128], ident_sb[:, :])
                        nc.tensor.transpose(
                            ptt[:, (2 * j + 1) * 128:(2 * j + 2) * 128],
                            qkns[1][:, j * 128:(j + 1) * 128], ident_sb[:, :])
                    dst = qkT[:, :].rearrange(
                        "p (b n c) -> p b n c", b=12, n=NP)[:, bp2:bp2 + 6, n, :]
                    src = ptt[:, :].rearrange("p (j c) -> p j c", j=6)
                    if tx_count[0] % 2 == 0:
                        nc.vector.tensor_copy(dst, src)
                    else:
                        nc.scalar.copy(dst, src)
                    tx_count[0] += 1
                return emit_transposes

            def attention_pair(p, sbe, ps2, ps3):
                for par in range(2):
                    h = 2 * p + par
                    et = [sbe.tile([128, N], BF16, tag=f"e{par}{nk}", name=f"e{par}{nk}") for nk in range(NP)]
                    qb, kb = (2 * p) * N, (2 * p + 1) * N
                    for nk in range(NP):
                        p2 = ps2.tile([128, 1024], F32, tag="m2", name="m2ps")
                        for nqh in range(2):
                            nc.tensor.matmul(
                                p2[:, nqh * 512:(nqh + 1) * 512],
                                qkT[par * D:(par + 1) * D, kb + nk * 128:kb + (nk + 1) * 128],
                                qkT[par * D:(par + 1) * D, qb + nqh * 512:qb + (nqh + 1) * 512],
                                start=True, stop=True)
                        nc.scalar.activation(
                            out=et[nk][:, :],
                            in_=p2[:, :], func=AF.Exp, scale=float(D) ** -0.5)
                    for nqh in range(2):
                        p3 = ps3.tile([D + 1, 512], F32, tag="m3", name="m3ps")
                        for nk in range(NP):
                            nc.tensor.matmul(
                                p3[:, :],
                                v1[nk][:, h, :],
                                et[nk][:, nqh * 512:(nqh + 1) * 512],
                                start=(nk == 0), stop=(nk == NP - 1))
                        sS = sbt.tile([1, 512], F32, tag="sS")
                        nc.vector.tensor_copy(sS[:, :], p3[D:D + 1, :])
                        rS = sbt.tile([1, 512], F32, tag="rS")
                        nc.vector.reciprocal_approx_fast(out=rS, in_=sS[:, :])
                        rSb = sbt.tile([D, 512], F32, tag="rSb")
                        nc.gpsimd.partition_broadcast(rSb[:, :], rS[:, :])
                        nc.vector.tensor_tensor(
                            out=aoT[p][par * D:(par + 1) * D,
                                       nqh * 512:(nqh + 1) * 512],
                            in0=p3[0:D, :], in1=rSb[:, :], op=ALU.mult)

            # ---------- phase 1: M1 + norm ----------
            ps1_cm = tc.tile_pool(name="ps1", bufs=6, space="PSUM")
            ps1 = ps1_cm.__enter__()
            psT_cm = tc.tile_pool(name="psT", bufs=2, space="PSUM")
            psT = psT_cm.__enter__()
            # v chunks interleave into the qk passes: v matmuls are PE-bound
            # with idle DVE/ACT, qk norm is DVE/ACT-bound with PE slack
            pending = None
            for fcv, (fcq, fck) in ((4, (0, 2)), (5, (1, 3))):
                for n in range(NP):
                    evac_v(fcv, n, m1_chunk(ps1, fcv, n))
                    w = evac_qk(ps1, psT, fcq, fck, n)
                    if pending is not None:
                        pending()
                    pending = w
            pending()
            psT_cm.__exit__(None, None, None)
            ps1_cm.__exit__(None, None, None)
            sbw_cm.__exit__(None, None, None)      # free x/w region for expT

            # ---------- phase 2: attention (exp-table phase) ----------
            sbe_cm = tc.tile_pool(name="sbe", bufs=2)
            sbe = sbe_cm.__enter__()
            ps2_cm = tc.tile_pool(name="ps2", bufs=2, space="PSUM")
            ps2 = ps2_cm.__enter__()
            ps3_cm = tc.tile_pool(name="ps3", bufs=4, space="PSUM")
            ps3 = ps3_cm.__enter__()
            for p in range(6):
                attention_pair(p, sbe, ps2, ps3)
            ps3_cm.__exit__(None, None, None)
            ps2_cm.__exit__(None, None, None)
            sbe_cm.__exit__(None, None, None)

            # ---------- phase 3: proj ----------
            psP_cm = tc.tile_pool(name="psP", bufs=4, space="PSUM")
            psP = psP_cm.__enter__()
            for n in range(NP):
                ysb = sbt.tile([128, C], F32, tag="y")
                for half in range(2):
                    pp = psP.tile([128, 384], F32, tag="mp", name="mpps")
                    for k in range(KC):
                        nc.tensor.matmul(
                            pp[:, :],
                            aoT[k][:, n * 128:(n + 1) * 128],
                            wp[k][:, half * 384:(half + 1) * 384],
                            start=(k == 0), stop=(k == KC - 1))
                    nc.vector.tensor_tensor(
                        out=ysb[:, half * 384:(half + 1) * 384], in0=pp[:, :],
                        in1=bias_sb[:, half * 384:(half + 1) * 384], op=ALU.add)
                    nc.sync.dma_start(
                        out=y_d[n * 128:(n + 1) * 128, half * 384:(half + 1) * 384],
                        in_=ysb[:, half * 384:(half + 1) * 384])
            psP_cm.__exit__(None, None, None)

    nc.compile()
    return nc


def _prep_inputs(x, qkv_w, proj_w, proj_b):
    import ml_dtypes
    wqkvT = np.ascontiguousarray(qkv_w.T).astype(ml_dtypes.bfloat16)    # [768, 2304]
    wpT = np.ascontiguousarray(proj_w.T).astype(ml_dtypes.bfloat16)     # [768, 768]
    bias = proj_b.reshape(1, C).astype(np.float32)
    ones16 = np.ones((128, 16), dtype=ml_dtypes.bfloat16)
    ident = np.eye(128, dtype=ml_dtypes.bfloat16)
    # q/k weight chunks augmented with per-head column sums (fc order 0,2,1,3)
    wqx = np.zeros((C, 4, 390), dtype=np.float64)
    for i, fc in enumerate((0, 2, 1, 3)):
        cols = qkv_w.T[:, fc * 384:(fc + 1) * 384].astype(np.float64)
        wqx[:, i, 0:384] = cols
        wqx[:, i, 384:390] = cols.reshape(C, 6, D).sum(-1)
    wqxT = np.ascontiguousarray(wqx.reshape(C, 4 * 390)).astype(ml_dtypes.bfloat16)
    maps = []
    for b in range(B):
        maps.append({
            "xT": np.ascontiguousarray(x[b].T).astype(ml_dtypes.bfloat16),
            "wqkvT": wqkvT, "wpT": wpT, "bias": bias, "ones16": ones16,
            "ident": ident, "wqxT": wqxT,
        })
    return maps


def kernel(x, qkv_w, proj_w, proj_b):
    from concourse.bass_utils import run_bass_kernel_spmd
    if "nc" not in _CACHE:
        _CACHE["nc"] = _build_nc()
    nc = _CACHE["nc"]
    maps = _prep_inputs(np.asarray(x), np.asarray(qkv_w),
                        np.asarray(proj_w), np.asarray(proj_b))
    import os
    trace = bool(os.environ.get("KERNEL_TRACE"))
    res = run_bass_kernel_spmd(nc, maps, list(range(B)), trace=trace)
    _CACHE["last_result"] = res
    out = np.stack([res.results[b]["y"] for b in range(B)], axis=0)
    return out.astype(np.float32)
